# revision 68
# baseline (speedup 1.0000x reference)
"""Trainium2 Bass kernel for nn_CombinatorialClassifierSplit.

Reference computation:
    xr = x.reshape(B, P, S)
    logits = einsum('bps,pks', xr, W) + b          # (B, P, K)
    logp = log_softmax(logits, axis=2)
    out[b, c] = sum_p logp[b, p, idx[p, c]]        # (B, C)

Key restructuring: since idx doesn't depend on b,
    out[b, c] = sum_p logits[b, p, idx[p, c]] - LSE[b]
with LSE[b] = sum_p logsumexp_k(logits[b, p, :]).  The first term is a
plain matmul  x_flat @ Wg + bsum[c]  where Wg[(p,s), c] = W[p, idx[p,c], s]
and bsum[c] = sum_p b[p, idx[p,c]] are host-side gathers of the *static*
index tensor.  Classes C are sharded 8 ways; each core runs:
  - per-p matmuls for logits -> exp -> segmented sum -> ln -> -LSE
  - one big fp8 DoubleRow matmul (contract 2048 as 8x256) over its C-shard
  - + bsum via a rank-1 matmul, - LSE adds, outputs written back (bf16)
    via prepared SWDGE kv_writeback descriptors fired by trigger_dma so the
    dependent tail skips the HWDGE descriptor-generation latency.
All heavy operands are fp8 (e4m3) to halve HBM/DMA traffic, which is the
bottleneck; accumulation stays fp32 in PSUM so the LSE path is accurate.
"""

import numpy as np
import ml_dtypes

import concourse.bacc as bacc
import concourse.tile as tile
from concourse import mybir
from concourse.bass_utils import run_bass_kernel_spmd

F8 = ml_dtypes.float8_e4m3
BF16 = ml_dtypes.bfloat16

B, P, K, S, C = 128, 32, 100, 64, 10000
N_CORES = 8
CS = C // N_CORES          # 1250 classes per core
NT2 = 8                    # 256-deep contraction chunk-pairs (DoubleRow)
CSP = 1280                 # padded SBUF stride for the c axis (16-aligned)
# psum tiles (c0, cn): two 625-wide DMA halves, each split 512+113
PSUM_TILES = [(0, 512), (512, 113), (625, 512), (1137, 113)]
# out write-back pieces (c0, cn, swdge queue); A half on q1, B half on q2
OUT_PIECES = [(0, 512, 1), (512, 113, 1), (625, 512, 2), (1137, 113, 2)]
# wg DMA pieces (j0, j1, c0, cn), emission order = rough transfer order.
# A half streams first so its psum tiles close early; the B tail is split
# progressively finer so the late matmuls start as soon as each piece
# lands.  Columns [1137:1250) travel in a separate c-padded tensor (wgt)
# whose pieces are fully contiguous — no sub-512B descriptor penalty —
# and form the short final dependent chain.
WG_PIECES = [
    (0, 4, 0, 625), (4, 8, 0, 625),
    (0, 4, 625, 512), (4, 6, 625, 512), (6, 7, 625, 512),
    (7, 8, 625, 512),
]
CT = 128                   # padded column count of the wgt tail tensor
# aux tensor layout: [bias (P*K) | bsum (CS) | ones (128)]
AUX_BIAS, AUX_BSUM, AUX_ONES = 0, P * K, P * K + CS
AUX_LEN = P * K + CS + 128

_cached = {}


def _build_program():
    if "nc" in _cached:
        return _cached["nc"]

    nc = bacc.Bacc("TRN2", target_bir_lowering=False, debug=False,
                   num_devices=N_CORES, num_swdge_queues=3)
    dt = mybir.dt
    DR = mybir.MatmulPerfMode.DoubleRow

    xt_d = nc.dram_tensor("xt", [128, NT2, 2, 128], dt.float8e4,
                          kind="ExternalInput")
    wg_d = nc.dram_tensor("wg", [128, NT2, 2, CS], dt.float8e4,
                          kind="ExternalInput")
    wgt_d = nc.dram_tensor("wgt", [128, NT2, 2, CT], dt.float8e4,
                           kind="ExternalInput")
    wk_d = nc.dram_tensor("wk", [128, 2 * NT2, K], dt.float8e4,
                          kind="ExternalInput")
    aux_d = nc.dram_tensor("aux", [1, AUX_LEN], dt.bfloat16,
                           kind="ExternalInput")
    out_d = nc.dram_tensor("out", [128, CS], dt.bfloat16,
                           kind="ExternalOutput")

    dma_sems = {q: nc.alloc_semaphore(f"out_wb_sem{q}") for q in (1, 2)}

    with tile.TileContext(nc) as tc:
        with (
            tc.tile_pool(name="const", bufs=1) as cpool,
            tc.tile_pool(name="psum_main", bufs=4, space="PSUM") as pmain,
            tc.tile_pool(name="psum_log", bufs=4, space="PSUM") as plog,
        ):
            xt_sb = cpool.tile([128, NT2, 2, 128], dt.float8e4)
            wk_sb = cpool.tile([128, 2 * NT2, K], dt.float8e4)
            aux_sb = cpool.tile([1, AUX_LEN], dt.bfloat16)
            wg_sb = cpool.tile([128, NT2, 2, CSP], dt.float8e4)
            wgt_sb = cpool.tile([128, NT2, 2, CT], dt.float8e4)
            exp_sb = cpool.tile([128, P, K], dt.bfloat16)
            sums_sb = cpool.tile([128, P], dt.bfloat16)
            lns_sb = cpool.tile([128, P], dt.float32)
            nlse_sb = cpool.tile([128, 1], dt.float32)
            dumln_sb = cpool.tile([1, 1], dt.float32)
            dumw_sb = cpool.tile([1, 128], dt.bfloat16)
            out_sb = cpool.tile([128, CS], dt.bfloat16)
            cidx_sb = cpool.tile([128, len(OUT_PIECES)], dt.int32)

            bias = lambda lo, n: aux_sb[:, AUX_BIAS + lo:AUX_BIAS + lo + n]
            bsum = lambda lo, n: aux_sb[:, AUX_BSUM + lo:AUX_BSUM + lo + n]
            ones_ap = aux_sb[:, AUX_ONES:AUX_ONES + 128]

            # ctx indices for the write-back descriptors (static constants)
            for i, (c0, cn, q) in enumerate(OUT_PIECES):
                nc.vector.memset(cidx_sb[:, i:i + 1], c0)

            # --- input DMAs.  xt then wk on the SP HWDGE queue followed by
            # the wg stream (Act's longer DGE delay would leave a bubble
            # after xt); aux via Pool SWDGE (no HWDGE slot at all, its 25ns
            # transfer slips into the stream).
            nc.sync.dma_start(xt_sb[:], xt_d[:])
            nc.sync.dma_start(wk_sb[:], wk_d[:])
            nc.gpsimd.dma_start(aux_sb[:], aux_d[:])
            for (j0, j1, c0, cn) in WG_PIECES:
                nc.sync.dma_start(wg_sb[:, j0:j1, :, c0:c0 + cn],
                                  wg_d[:, j0:j1, :, c0:c0 + cn])
            nc.sync.dma_start(wgt_sb[:, 0:6], wgt_d[:, 0:6])
            nc.sync.dma_start(wgt_sb[:, 6:NT2], wgt_d[:, 6:NT2])

            # prepared write-back descriptors.  Desc-gen runs early on the
            # idle Pool engine (the prep only sync-depends on the cidx
            # memsets); the adds -> DMA ordering is enforced by the
            # triggers' signals_writable edges below.  out viewed
            # [batch=1, dhi=128, dho=1, n_ctx=CS]; in viewed
            # [dhi=128, dho=1, batch=1, ncn].  The singleton dims are never
            # walked, but the ucode stride decode needs dho_stride == n_ctx
            # on the out side and an ncn-divisible (0) dho stride on the in
            # side.
            preps = []
            for i, (c0, cn, q) in enumerate(OUT_PIECES):
                o4 = out_d[:].unsqueeze(0).unsqueeze(2)
                o4.ap[2] = (CS, 1)
                i4 = out_sb[:, c0:c0 + cn].unsqueeze(1).unsqueeze(2)
                i4.ap[1] = (0, 1)
                preps.append(nc.gpsimd.kv_writeback(
                    o4, i4, cidx_sb[:, i:i + 1],
                    prepare_only=True, sem=dma_sems[q], queue_num=q))

            # dummy Ln so the Exp+Ln activation table loads up front instead
            # of in the middle of the LSE chain
            nc.scalar.activation(dumln_sb[:], ones_ap[0:1, 0:1],
                                 mybir.ActivationFunctionType.Ln)

            # tiny dummy matmuls on garbage data: they pin the Tensor
            # engine's activity window early so the HAM clock gate is fully
            # open (2.4 GHz) by the time the real matmuls arrive
            dum_ps = pmain.tile([128, 16], dt.float32, tag="mm")
            nc.vector.memset(dumw_sb[:], 0)
            for _ in range(2):
                nc.tensor.matmul(dum_ps[:, 0:16], dumw_sb[:, 0:128],
                                 dumw_sb[:, 0:16], start=True, stop=True)

            # --- logits -> exp -> per-p sums (each psum tile holds 4 p's) ---
            for j in range(P // 4):
                ps = plog.tile([128, 4 * K], dt.float32, tag="lg")
                for q in range(4):
                    p = 4 * j + q
                    t, h = p // 2, p % 2
                    reg = ps[:, q * K:(q + 1) * K]
                    nc.tensor.matmul(reg, ones_ap, bias(p * K, K),
                                     start=True, stop=False)
                    nc.tensor.matmul(reg,
                                     xt_sb[h * 64:h * 64 + 64, p // 4,
                                           (p % 4) // 2, :],
                                     wk_sb[h * 64:h * 64 + 64, t, :],
                                     start=False, stop=True)
                nc.scalar.activation(exp_sb[:, 4 * j:4 * j + 4, :],
                                     ps[:, 0:4 * K],
                                     mybir.ActivationFunctionType.Exp)
                with nc.allow_low_precision(reason="bf16 exp sums; ln is "
                                            "tolerant of 0.4% rel err"):
                    nc.vector.tensor_reduce(sums_sb[:, 4 * j:4 * j + 4],
                                            exp_sb[:, 4 * j:4 * j + 4, :],
                                            axis=mybir.AxisListType.X,
                                            op=mybir.AluOpType.add)

            # --- LSE ---
            nc.scalar.activation(lns_sb[:], sums_sb[:],
                                 mybir.ActivationFunctionType.Ln)
            nc.vector.tensor_reduce(nlse_sb[:], lns_sb[:],
                                    axis=mybir.AxisListType.X,
                                    op=mybir.AluOpType.add, negate=True)

            # --- main matmul over C-shard: fp8 DoubleRow, 8 chunks of 256;
            # tile-outer so the A half's psum tiles close as soon as the A
            # wg pieces land, long before the B tail arrives.  bsum inits
            # sit after the logits groups so they don't stall the LSE chain
            # on the aux DMA. ---
            pts = []
            for (c0, cn) in PSUM_TILES:
                ps = pmain.tile([128, cn], dt.float32, tag="mm")
                pts.append(ps)
                nc.tensor.matmul(ps[:, 0:cn], ones_ap, bsum(c0, cn),
                                 start=True, stop=False)
            for ps, (c0, cn) in zip(pts, PSUM_TILES):
                for j in range(NT2):
                    rhs = (wgt_sb[:, j, :, 0:cn] if c0 == 1137
                           else wg_sb[:, j, :, c0:c0 + cn])
                    nc.tensor.matmul(ps[:, 0:cn],
                                     xt_sb[:, j, :, :], rhs,
                                     start=False, stop=(j == NT2 - 1),
                                     perf_mode=DR)

            # --- -LSE adds (DVE; Act's Identity shares the preloaded table
            # but DVE keeps Act free for the exp stream) + triggered
            # write-back per half ---
            from concourse.instruction_name_ordered_set import (
                InstructionNameOrderedSet)

            def nosync(inst, others):
                deps = InstructionNameOrderedSet()
                for o in others:
                    deps.add(o.ins.name)
                inst.ins.add_nosync_dependencies_from(deps)

            # -LSE adds spread across DVE / Act / Pool so each half's adds
            # run in parallel and its trigger fires sooner (Act's
            # Identity+bias shares the preloaded table).  The B half's wide
            # add is split across DVE+Act because it sits on the critical
            # tail.
            def add_act(c0, cn, ps, p0):
                nc.scalar.activation(out_sb[:, c0:c0 + cn], ps[:, p0:p0 + cn],
                                     mybir.ActivationFunctionType.Identity,
                                     bias=nlse_sb[:])

            def add_dve(c0, cn, ps, p0):
                nc.vector.tensor_scalar_add(out_sb[:, c0:c0 + cn],
                                            ps[:, p0:p0 + cn], nlse_sb[:])

            add_act(0, 512, pts[0], 0)
            add_dve(512, 113, pts[1], 0)
            trigs = {1: nc.gpsimd.trigger_dma(
                count=None, queue_num=1,
                signals_writable=[out_sb[:, 0:625]])}
            add_act(625, 512, pts[2], 0)
            add_dve(1137, 113, pts[3], 0)
            trigs[2] = nc.gpsimd.trigger_dma(
                count=None, queue_num=2,
                signals_writable=[out_sb[:, 625:CS]])
            # keep every prep ahead of both triggers in the Pool stream, and
            # each completion wait behind its trigger
            for q in (1, 2):
                nosync(trigs[q], preps)
                w = nc.gpsimd.wait_ge(dma_sems[q], 16 * 2)
                nosync(w, [trigs[q]])

    nc.compile()

    # Tile's sem-assignment gives each PREPARE_ONLY SWDGE prep a DMASW lane
    # tick, but nothing ever increments that lane sem for preps (the DMA
    # completion fires the prep's own sem= instead) — so the end-of-program
    # drain eventsems would wait forever, on the simulator and on hardware
    # alike.  Strip waits on semaphores that no instruction updates; actual
    # write-back completion is enforced by the explicit wait_ge on the
    # out_wb sems above.
    # The act-table pass inserts one minimal-set load per need (Ln -> Exp ->
    # Ln again), the last of which lands in the middle of the LSE chain.
    # One combined Exp+Ln+Identity table serves every activation here, so
    # retarget the first load at it and drop the rest.
    from concourse.hw_specs import get_activation_tables
    table_names = list(get_activation_tables(nc.m.arch))
    combined_id = table_names.index("natural_log_exp_and_others")
    seen_load = False
    for blk in nc.m.functions[0].blocks:
        for inst in list(blk.instructions):
            if isinstance(inst, mybir.InstLoadActFuncSet):
                si = inst.sync_info
                assert not (si and (si.on_wait or si.on_update))
                if not seen_load:
                    inst.act_func_set_id = combined_id
                    seen_load = True
                else:
                    blk.instructions.remove(inst)

    # The tile entry barrier delays the first DMA by ~0.7us and the two
    # epilogue barrier rounds add ~0.6us after the final write-back sem.
    # Every cross-engine ordering this kernel needs is carried by data
    # semaphores (the explicit out_wb waits cover the final DMAs), so the
    # all-engine barriers are pure overhead — drop every instruction that
    # touches the barrier semaphore.
    def _refs_barrier(si):
        if not si:
            return False
        for w in list(si.on_wait) + list(si.on_update):
            if w.ant_name and w.ant_name.startswith("barrier_Pool"):
                return True
        return False

    for blk in nc.m.functions[0].blocks:
        for inst in list(blk.instructions):
            if _refs_barrier(inst.sync_info):
                blk.instructions.remove(inst)

    # Pool's block-terminating branch runs after the final out_wb wait and
    # costs 61ns on the critical end; hoist the wait across the branch into
    # the end block (Pool per-engine order is preserved: the wait still
    # precedes the semaphore range-clear).  Also drop SP's epilogue wait on
    # the trigger sequencer ticks: the cost model routes trigger updates
    # through the +900ns DMA path, but completion is already enforced by the
    # Pool-side out_wb waits.
    blocks = list(nc.m.functions[0].blocks)
    tile_blk = blocks[1]
    end_blk = blocks[2]
    for inst in list(tile_blk.instructions):
        si = inst.sync_info
        if (isinstance(inst, mybir.InstEventSemaphore) and si and any(
                w.ant_name and w.ant_name.startswith("out_wb_sem")
                for w in si.on_wait)):
            tile_blk.instructions.remove(inst)
            end_blk.instructions.insert(0, inst)
    for inst in end_blk.instructions:
        si = inst.sync_info
        if isinstance(inst, mybir.InstEventSemaphore) and si and si.on_wait:
            kept = [w for w in si.on_wait
                    if not (w.ant_name
                            and w.ant_name.startswith("Pool_sequencer"))]
            if len(kept) != len(si.on_wait):
                si.on_wait = kept

    # The epilogue's bare engine drains are no-ops by construction here:
    # every engine's last real op is upstream of a DMA-completion semaphore
    # we explicitly wait on, so the pipelines are long empty.  Keep drains
    # that carry sem waits (none today) and the semaphore range-clear (it
    # resets state for repeat dispatches).
    for blk in nc.m.functions[0].blocks:
        if not blk.name.endswith("_end"):
            continue
        for inst in list(blk.instructions):
            si = inst.sync_info
            if isinstance(inst, mybir.InstDrain) and not (
                    si and (si.on_wait or si.on_update)):
                blk.instructions.remove(inst)

    # Merge the three-block CFG into one: with the barriers gone the blocks
    # chain through unconditional per-engine branches only, and SP's entry
    # branch costs 50ns ahead of the first DMA's descriptor-gen.  Branch
    # waits (the fused out_wb_sem1 wait on Pool's tile-exit branch) move to
    # the next same-engine instruction.
    blocks2 = list(nc.m.functions[0].blocks)
    mainb = blocks2[0]
    pending_waits = {}
    merged = []
    for blk in blocks2:
        for inst in list(blk.instructions):
            if isinstance(inst, mybir.InstUnconditionalBranch):
                si = inst.sync_info
                if si and si.on_wait:
                    pending_waits.setdefault(inst.engine, []).extend(si.on_wait)
                continue
            if inst.engine in pending_waits:
                w = pending_waits.pop(inst.engine)
                si = inst.sync_info
                if si is None:
                    inst.sync_info = mybir.SyncInfo(on_wait=w, on_update=[])
                else:
                    si.on_wait = list(si.on_wait) + w
            merged.append(inst)
    assert not pending_waits, pending_waits
    for blk in blocks2:
        blk.instructions.clear()
    for inst in merged:
        mainb.instructions.append(inst)

    updated = set()
    for blk in nc.m.functions[0].blocks:
        for inst in blk.instructions:
            si = inst.sync_info
            if si:
                for u in si.on_update:
                    updated.add(u.ant_name)
    for blk in nc.m.functions[0].blocks:
        for inst in blk.instructions:
            si = inst.sync_info
            if si and si.on_wait:
                kept = [w for w in si.on_wait
                        if not (w.ant_name and w.ant_name.startswith("DMASW")
                                and w.ant_name not in updated)]
                if len(kept) != len(si.on_wait):
                    si.on_wait = kept

    _cached["nc"] = nc
    return nc


def _prep_inputs(x, W, b, idx):
    """Host-side data prep -> per-core input maps."""
    x = np.asarray(x, dtype=np.float32)
    W = np.asarray(W, dtype=np.float32)
    b = np.asarray(b, dtype=np.float32)
    idx = np.asarray(idx, dtype=np.int64)

    # x^T in (row, chunk-pair j, k, b) layout: flat (p,s) = (j*2+k)*128 + row
    xt = np.ascontiguousarray(
        x.T.reshape(NT2, 2, 128, B).transpose(2, 0, 1, 3)).astype(F8)

    # packed per-pair weights for the logits path: (128, 16, K)
    wk = np.empty((128, 2 * NT2, K), dtype=np.float32)
    for t in range(2 * NT2):
        wk[0:64, t, :] = W[2 * t].T
        wk[64:128, t, :] = W[2 * t + 1].T
    wk = wk.astype(F8)

    # gathered big weight matrix: Wg[(p,s), c] = W[p, idx[p,c], s]
    Wg = W[np.arange(P)[:, None], idx]            # (P, C, S)
    Wg = np.ascontiguousarray(Wg.transpose(0, 2, 1)).reshape(P * S, C)
    bsum_full = b[np.arange(P)[:, None], idx].sum(axis=0)   # (C,)

    aux_base = np.zeros((1, AUX_LEN), dtype=np.float32)
    aux_base[0, AUX_BIAS:AUX_BIAS + P * K] = b.reshape(-1)
    aux_base[0, AUX_ONES:AUX_ONES + 128] = 1.0

    in_maps = []
    for m in range(N_CORES):
        sl = Wg[:, m * CS:(m + 1) * CS]
        wg = np.ascontiguousarray(
            sl.reshape(NT2, 2, 128, CS).transpose(2, 0, 1, 3)).astype(F8)
        wgt = np.zeros((128, NT2, 2, CT), dtype=F8)
        wgt[:, :, :, 0:CS - 1137] = wg[:, :, :, 1137:CS]
        aux = aux_base.copy()
        aux[0, AUX_BSUM:AUX_BSUM + CS] = bsum_full[m * CS:(m + 1) * CS]
        in_maps.append({"xt": xt, "wg": wg, "wgt": wgt, "wk": wk,
                        "aux": aux.astype(BF16)})
    return in_maps


def kernel(x, W, b, partitionings):
    nc = _build_program()
    in_maps = _prep_inputs(x, W, b, partitionings)
    res = run_bass_kernel_spmd(nc, in_maps, list(range(N_CORES)))
    out = np.concatenate([np.asarray(res.results[m]["out"])
                          for m in range(N_CORES)], axis=1)
    return out.astype(np.float32)
